# revision 27
# baseline (speedup 1.0000x reference)
"""Multi-head attention (B=2, N=2048, C=1024, H=16) on 8 Trainium2 cores.

Sharding: core cid = (b, hg) with b = cid//4, hg = cid%4.  Data-parallel on
batch, 4-way tensor-parallel on heads (4 heads / 256 dims per core).  Each
core computes q/k/v projections for its head slice, full (masked-softmax)
attention for its 4 heads, and a partial output projection y^T = Wp_slice^T
-contracted over its 256 dims.  Host sums the 4 partials per batch and adds
the proj bias.

v4 pipeline:
  - Everything bf16 on-device (f32 PSUM accumulation); host pre-casts x, W,
    and the mask (no on-device uint8->bf16 casts).
  - Block (hp, ncb) = 16-m-tile sweep: scores S^T = K^T_h x Q^T_h (head
    pair packed at partition offsets 0/64), exp on Scalar (scale folded),
    mask multiply on DVE into a fresh tile, PV lag-4 behind scores.
  - Step-granular boundary interleave: the last 4 steps of block P
    alternate with the first 4 steps of block B, so the Scalar engine's
    exp stream never drains at block handoffs; P's PV flush + normalize
    land behind B's early steps as PE fill.
  - proj-C psy pairs (2-bank PSUM tiles) spread through the following
    block's steps; psy copy-out split Scalar/DVE.
  - PSUM: pool2 3x2-bank (phase-A segments / score ping-pong / psy pairs)
    + pool1 2x1-bank (O^T accumulators) = 8 banks.
"""

import os
import sys
import types
from contextlib import ExitStack

import numpy as np
import ml_dtypes

import concourse.bass as bass
import concourse.mybir as mybir
import concourse.tile as tile
from concourse import bacc
from concourse.bass_utils import run_bass_kernel_spmd

# ---------------------------------------------------------------- constants
N = 2048          # sequence length
C = 1024          # model dim
NH = 4            # heads per core
HD = 64           # head dim
DQK = 2 * NH * HD # 512: q rows then k rows in qk^T
DV = NH * HD      # 256
NCK = 512         # n-chunk size
NCH = N // NCK    # 4 n-chunks
MT = N // 128     # 16 m-tiles
CK = C // 128     # 8 contraction chunks
SCALE = HD ** -0.5
NCORES = 8

F32 = mybir.dt.float32
BF16 = mybir.dt.bfloat16


def _ensure_ntff_hook():
    """bass_utils' trace path imports antenv.axon_hooks, which this image
    lacks; inject it and register the ctypes-based NTFF profile hook."""
    if "antenv.axon_hooks" in sys.modules:
        return
    mod = types.ModuleType("antenv.axon_hooks")
    _hook = [None]
    mod.set_axon_ntff_profile_hook = lambda h: _hook.__setitem__(0, h)
    mod.get_axon_ntff_profile_hook = lambda: _hook[0]
    sys.modules["antenv.axon_hooks"] = mod
    try:
        from trn_agent_boot.trn_boot import _ntff_profile_via_ctypes

        mod.set_axon_ntff_profile_hook(
            _ntff_profile_via_ctypes("/opt/axon/libaxon_pjrt.so")
        )
    except Exception:
        pass


def build():
    nc = bacc.Bacc("TRN2", target_bir_lowering=False, debug=False,
                   num_devices=NCORES)
    xT = nc.dram_tensor("xT", [C, N], BF16, kind="ExternalInput")
    wqk = nc.dram_tensor("wqkT", [C, DQK], BF16, kind="ExternalInput")
    wv = nc.dram_tensor("wvT", [C, DV], BF16, kind="ExternalInput")
    wp = nc.dram_tensor("wpT", [DV, C], BF16, kind="ExternalInput")
    mk = nc.dram_tensor("maskT", [N, N], BF16, kind="ExternalInput")
    yT = nc.dram_tensor("yT", [C, N], F32, kind="ExternalOutput")

    with tile.TileContext(nc) as tc, ExitStack() as ctx:
        consts = ctx.enter_context(tc.tile_pool(name="consts", bufs=1))
        xin = ctx.enter_context(tc.tile_pool(name="xin", bufs=16))
        ptp = ctx.enter_context(tc.tile_pool(name="ptp", bufs=4))
        pt2p = ctx.enter_context(tc.tile_pool(name="pt2p", bufs=24))
        ysb = ctx.enter_context(tc.tile_pool(name="ysb", bufs=2))
        dnp = ctx.enter_context(tc.tile_pool(name="dnp", bufs=2))
        rbp = ctx.enter_context(tc.tile_pool(name="rbp", bufs=2))
        # PSUM: 3x 2-bank + 2x 1-bank tiles = 8 banks static.
        pool2 = ctx.enter_context(tc.tile_pool(name="pool2", bufs=3, space="PSUM"))
        pool1 = ctx.enter_context(tc.tile_pool(name="pool1", bufs=2, space="PSUM"))

        # ---- resident weights (wp loaded late, after phase-A x traffic)
        wqk_sb = consts.tile([128, CK, DQK], BF16)
        wv_sb = consts.tile([128, CK, DV], BF16)
        wp_sb = consts.tile([128, 2, C], BF16)
        nc.sync.dma_start(out=wqk_sb,
                          in_=wqk[:].rearrange("(co ci) d -> ci co d", ci=128))

        # ---- per-chunk / per-mtile intermediates (fine-grained deps)
        qkT = [[consts.tile([128, NCK], BF16, name=f"qk_m{m}_c{c}")
                for c in range(NCH)] for m in range(4)]
        vb = [consts.tile([128, NH, HD + 1], BF16, name=f"vb_{mt}")
              for mt in range(MT)]
        mask_sb = [consts.tile([128, N], BF16, name=f"mask_m{mt}")
                   for mt in range(MT)]
        ot_sb = [consts.tile([128, 2, NCK], BF16, name=f"ot_n{ncb}")
                 for ncb in range(NCH)]

        # V_aug ones column (during DMA head; engines idle)
        for mt in range(MT):
            nc.gpsimd.memset(vb[mt][:, :, HD:HD + 1], 1.0)

        warm = consts.tile([128, NCK], BF16, name="warm")
        nc.vector.memset(warm[:, 0:NCK], 0.0)

        def emit_warmup(n):
            pwarm = pool1.tile([128, NCK], F32, tag="p1", name="pwarm")
            for i in range(n):
                nc.tensor.matmul(pwarm, lhsT=warm[:, 0:128], rhs=warm,
                                 start=True, stop=True)

        def emit_mask(mts):
            for mt in mts:
                nc.sync.dma_start(out=mask_sb[mt],
                                  in_=mk[mt * 128:(mt + 1) * 128, :])

        def emit_x_dma(c):
            xts = []
            for cc in range(CK):
                xt = xin.tile([128, NCK], BF16)
                nc.sync.dma_start(
                    out=xt[:, 0:NCK // 2],
                    in_=xT[cc * 128:(cc + 1) * 128,
                           c * NCK:c * NCK + NCK // 2])
                nc.sync.dma_start(
                    out=xt[:, NCK // 2:],
                    in_=xT[cc * 128:(cc + 1) * 128,
                           c * NCK + NCK // 2:(c + 1) * NCK])
                xts.append(xt)
            return xts

        # ---------------- phase A: q/k segments (paired so hp=0's Q+K
        # tiles land first), then v segments; PSUM->SBUF copies on
        # Scalar/DVE (early phase, exp stream is light)
        def emit_A_qk(c, xts, pair):
            ms = (0, 2) if pair == 0 else (1, 3)
            pa = pool2.tile([128, 2, NCK], F32, tag="p2")
            for cc in range(CK):
                for mm, m in enumerate(ms):
                    nc.tensor.matmul(
                        pa[:, mm, :],
                        lhsT=wqk_sb[:, cc, m * 128:(m + 1) * 128],
                        rhs=xts[cc], start=(cc == 0), stop=(cc == CK - 1))
            for mm, m in enumerate(ms):
                if mm == 0:
                    nc.scalar.copy(out=qkT[m][c], in_=pa[:, mm, :])
                else:
                    nc.vector.tensor_copy(out=qkT[m][c], in_=pa[:, mm, :])

        def emit_A_v(c, xts, seg):
            pv = pool2.tile([128, 2, NCK], F32, tag="p2")
            for cc in range(CK):
                for jj in range(2):
                    j = 2 * seg + jj
                    nc.tensor.matmul(
                        pv[:, jj, 0:DV],
                        lhsT=xts[cc][:, j * 128:(j + 1) * 128],
                        rhs=wv_sb[:, cc, :],
                        start=(cc == 0), stop=(cc == CK - 1))
            for jj in range(2):
                mt = 4 * c + 2 * seg + jj
                src = pv[:, jj, 0:DV].rearrange("p (h d) -> p h d", h=NH)
                if jj == 0:
                    nc.scalar.copy(out=vb[mt][:, :, 0:HD], in_=src)
                else:
                    nc.vector.tensor_copy(out=vb[mt][:, :, 0:HD], in_=src)

        # ---------------- attention block (hp, ncb)
        class Block:
            def __init__(self, hp, ncb):
                self.hp, self.ncb = hp, ncb
                self.nsl = slice(ncb * NCK, (ncb + 1) * NCK)
                self.pso = None
                self.pts = {}     # mt -> masked-P tile awaiting PV
                self.next_pv = 0
                self.next_step = 0

            def step1(self):
                if self.next_step < MT:
                    self.steps([self.next_step])

            def steps(self, mts):
                mq, mkt = self.hp, 2 + self.hp
                for mt in mts:
                    self.next_step = mt + 1
                    pss = pool2.tile([128, 2, NCK], F32, tag="p2")
                    for par in range(2):
                        po = par * 64
                        nc.tensor.matmul(
                            pss[:, par, :],
                            lhsT=qkT[mkt][mt // 4][po:po + 64,
                                                   (mt % 4) * 128:
                                                   (mt % 4 + 1) * 128],
                            rhs=qkT[mq][self.ncb][po:po + 64, :],
                            start=True, stop=True)
                    pt = ptp.tile([128, 2, NCK], BF16)
                    nc.scalar.activation(
                        out=pt, in_=pss,
                        func=mybir.ActivationFunctionType.Exp, scale=SCALE)
                    pt2 = pt2p.tile([128, 2, NCK], BF16)
                    for par in range(2):
                        nc.vector.tensor_mul(out=pt2[:, par, :],
                                             in0=pt[:, par, :],
                                             in1=mask_sb[mt][:, self.nsl])
                    self.pts[mt] = pt2

            def pvs(self, n):
                if self.pso is None:
                    self.pso = [pool1.tile([128, NCK], F32, tag="p1",
                                           name=f"pso{self.hp}_{self.ncb}_{p}")
                                for p in range(2)]
                for _ in range(n):
                    mt = self.next_pv
                    pt2 = self.pts.pop(mt)
                    for par in range(2):
                        nc.tensor.matmul(
                            self.pso[par][0:HD + 1, :],
                            lhsT=vb[mt][:, 2 * self.hp + par, :],
                            rhs=pt2[:, par, :],
                            start=(mt == 0), stop=(mt == MT - 1))
                    self.next_pv += 1

            def finalize(self):
                for par in range(2):
                    po = par * 64
                    den = dnp.tile([1, NCK], F32, tag="den")
                    nc.vector.tensor_copy(out=den,
                                          in_=self.pso[par][HD:HD + 1, :])
                    rec = dnp.tile([1, NCK], F32, tag="rec")
                    nc.vector.reciprocal_approx_fast(out=rec, in_=den)
                    rb = rbp.tile([64, NCK], F32)
                    nc.gpsimd.partition_broadcast(rb, rec)
                    nc.vector.tensor_mul(
                        out=ot_sb[self.ncb][po:po + 64, self.hp, :],
                        in0=self.pso[par][0:HD, :], in1=rb)

        # ---------------- output projection piece: 2 et rows per call
        def emit_proj_piece(ncb, pair):
            nsl = slice(ncb * NCK, (ncb + 1) * NCK)
            psy = pool2.tile([128, 2, NCK], F32, tag="p2")
            for ee in range(2):
                et = 2 * pair + ee
                for dk in range(2):
                    nc.tensor.matmul(
                        psy[:, ee, :],
                        lhsT=wp_sb[:, dk, et * 128:(et + 1) * 128],
                        rhs=ot_sb[ncb][:, dk, :],
                        start=(dk == 0), stop=(dk == 1))
            yt = ysb.tile([128, 2, NCK], F32)
            nc.vector.tensor_copy(out=yt[:, 0, :], in_=psy[:, 0, :])
            nc.vector.tensor_copy(out=yt[:, 1, :], in_=psy[:, 1, :])
            for ee in range(2):
                et = 2 * pair + ee
                nc.sync.dma_start(out=yT[et * 128:(et + 1) * 128, nsl],
                                  in_=yt[:, ee, :])

        # ---------------- schedule: one block step per phase-A segment
        # (keeps each score's exp dependency a full segment away -> the PE
        # never waits mid-phase-A and holds its high p-state)
        b0 = Block(0, 0)
        b1 = Block(1, 0)
        b01 = Block(0, 1)
        xts = emit_x_dma(0)
        emit_mask([0, 1])
        nc.sync.dma_start(out=wv_sb,
                          in_=wv[:].rearrange("(co ci) d -> ci co d", ci=128))
        emit_warmup(24)
        emit_mask([2, 3])
        emit_A_qk(0, xts, 0)
        b0.step1()
        emit_A_qk(0, xts, 1)
        b0.step1()
        b1.step1()
        emit_A_v(0, xts, 0)
        b0.step1()
        emit_A_v(0, xts, 1)
        b1.step1()
        nxts = emit_x_dma(1)
        emit_mask([4, 5, 6, 7])
        xts = nxts
        emit_A_qk(1, xts, 0)
        b0.step1()
        emit_A_qk(1, xts, 1)
        b0.step1()
        emit_A_v(1, xts, 0)
        b0.pvs(2)
        b0.step1()
        emit_A_v(1, xts, 1)
        b1.step1()
        b01.step1()
        nxts = emit_x_dma(2)
        emit_mask([8, 9, 10, 11])
        xts = nxts
        emit_A_qk(2, xts, 0)
        b0.step1()
        emit_A_qk(2, xts, 1)
        b0.step1()
        b0.pvs(2)
        emit_A_v(2, xts, 0)
        b0.step1()
        emit_A_v(2, xts, 1)
        b1.step1()
        b1.step1()
        b01.step1()
        nxts = emit_x_dma(3)
        emit_mask([12, 13, 14, 15])
        xts = nxts
        emit_A_qk(3, xts, 0)
        b0.step1()
        emit_A_qk(3, xts, 1)
        b0.step1()
        b0.pvs(2)
        emit_A_v(3, xts, 0)
        b0.step1()
        emit_A_v(3, xts, 1)
        b1.step1()
        b0.pvs(2)
        nc.sync.dma_start(out=wp_sb,
                          in_=wp[:].rearrange("(dk ci) e -> ci dk e", ci=128))

        # post-A steady pipeline: step-interleaved boundaries + proj fills
        order = [(1, 0), (0, 1), (1, 1), (0, 2), (1, 2), (0, 3), (1, 3)]
        blocks = {(1, 0): b1, (0, 1): b01}
        prev = b0
        pending_proj = []
        for hp, ncb in order:
            blk = blocks.get((hp, ncb)) or Block(hp, ncb)
            # boundary: prev's last steps alternate with blk's next steps;
            # prev's remaining PVs drain 2 per slot
            for i in range(4):
                prev.step1()
                blk.step1()
                prev.pvs(2)
            # finalize is delayed 3 steps: its DVE ops (normalize TTs
            # waiting on the gpsimd broadcasts) would otherwise head-block
            # the new block's mask stream at the boundary
            for i in range(8):
                blk.step1()
                if i == 2:
                    prev.finalize()
                    if prev.hp == 1:
                        pending_proj += [(prev.ncb, pair)
                                         for pair in range(4)]
                if pending_proj and i in (3, 5, 7):
                    emit_proj_piece(*pending_proj.pop(0))
                if i >= 3:
                    blk.pvs(2 if i <= 5 else 1)
            prev = blk
        # tail: last block steps 12..15, drain, finalize, proj(3)
        for i in range(4):
            prev.step1()
            prev.pvs(2)
        twarm = pool2.tile([128, 2, NCK], F32, tag="p2")
        for i in range(8):
            nc.tensor.matmul(twarm[:, 0, :], lhsT=warm[:, 0:128], rhs=warm,
                             start=True, stop=True)
        prev.finalize()
        for pair in range(4):
            emit_proj_piece(prev.ncb, pair)

    nc.compile()
    return nc


_NC = None


def _get_nc():
    global _NC
    if _NC is None:
        _NC = build()
    return _NC


def make_in_maps(x, mask, W_qkv, W_proj):
    x = np.asarray(x, dtype=np.float32)
    mask = np.asarray(mask)
    W_qkv = np.asarray(W_qkv, dtype=np.float32)
    W_proj = np.asarray(W_proj, dtype=np.float32)
    bf = ml_dtypes.bfloat16
    in_maps = []
    for cid in range(NCORES):
        b, hg = divmod(cid, 4)
        rs = slice(hg * 256, (hg + 1) * 256)
        wq = W_qkv[0 * C:1 * C][rs]          # [256, 1024]
        wk = W_qkv[1 * C:2 * C][rs]
        wvs = W_qkv[2 * C:3 * C][rs]
        in_maps.append({
            "xT": np.ascontiguousarray(x[b].T).astype(bf),
            "wqkT": np.ascontiguousarray(
                np.concatenate([wq, wk], axis=0).T).astype(bf),
            "wvT": np.ascontiguousarray(wvs.T).astype(bf),
            "wpT": np.ascontiguousarray(W_proj[:, rs].T).astype(bf),
            "maskT": np.ascontiguousarray(mask[b, 0].T).astype(bf),
        })
    return in_maps


LAST_EXEC_NS = None
LAST_MEAN_EXEC_NS = None


def kernel(x, mask, W_qkv, W_proj, b_proj):
    global LAST_EXEC_NS, LAST_MEAN_EXEC_NS
    trace = bool(int(os.environ.get("TRNK_TRACE", "0")))
    if trace:
        _ensure_ntff_hook()
    nc = _get_nc()
    in_maps = make_in_maps(x, mask, W_qkv, W_proj)
    res = run_bass_kernel_spmd(nc, in_maps, list(range(NCORES)), trace=trace)
    LAST_EXEC_NS = res.exec_time_ns
    LAST_MEAN_EXEC_NS = res.mean_exec_time_ns
    y = np.zeros((2, N, C), dtype=np.float32)
    for cid in range(NCORES):
        b = cid // 4
        y[b] += np.asarray(res.results[cid]["yT"], dtype=np.float32).T
    y += np.asarray(b_proj, dtype=np.float32)[None, None, :]
    return y


# revision 28
# speedup vs baseline: 1.0130x; 1.0130x over previous
"""Multi-head attention (B=2, N=2048, C=1024, H=16) on 8 Trainium2 cores.

Sharding: core cid = (b, hg) with b = cid//4, hg = cid%4.  Data-parallel on
batch, 4-way tensor-parallel on heads (4 heads / 256 dims per core).  Each
core computes q/k/v projections for its head slice, full (masked-softmax)
attention for its 4 heads, and a partial output projection y^T = Wp_slice^T
-contracted over its 256 dims.  Host sums the 4 partials per batch and adds
the proj bias.

v4 pipeline:
  - Everything bf16 on-device (f32 PSUM accumulation); host pre-casts x, W,
    and the mask (no on-device uint8->bf16 casts).
  - Block (hp, ncb) = 16-m-tile sweep: scores S^T = K^T_h x Q^T_h (head
    pair packed at partition offsets 0/64), exp on Scalar (scale folded),
    mask multiply on DVE into a fresh tile, PV lag-4 behind scores.
  - Step-granular boundary interleave: the last 4 steps of block P
    alternate with the first 4 steps of block B, so the Scalar engine's
    exp stream never drains at block handoffs; P's PV flush + normalize
    land behind B's early steps as PE fill.
  - proj-C psy pairs (2-bank PSUM tiles) spread through the following
    block's steps; psy copy-out split Scalar/DVE.
  - PSUM: pool2 3x2-bank (phase-A segments / score ping-pong / psy pairs)
    + pool1 2x1-bank (O^T accumulators) = 8 banks.
"""

import os
import sys
import types
from contextlib import ExitStack

import numpy as np
import ml_dtypes

import concourse.bass as bass
import concourse.mybir as mybir
import concourse.tile as tile
from concourse import bacc
from concourse.bass_utils import run_bass_kernel_spmd

# ---------------------------------------------------------------- constants
N = 2048          # sequence length
C = 1024          # model dim
NH = 4            # heads per core
HD = 64           # head dim
DQK = 2 * NH * HD # 512: q rows then k rows in qk^T
DV = NH * HD      # 256
NCK = 512         # n-chunk size
NCH = N // NCK    # 4 n-chunks
MT = N // 128     # 16 m-tiles
CK = C // 128     # 8 contraction chunks
SCALE = HD ** -0.5
NCORES = 8

F32 = mybir.dt.float32
BF16 = mybir.dt.bfloat16


def _ensure_ntff_hook():
    """bass_utils' trace path imports antenv.axon_hooks, which this image
    lacks; inject it and register the ctypes-based NTFF profile hook."""
    if "antenv.axon_hooks" in sys.modules:
        return
    mod = types.ModuleType("antenv.axon_hooks")
    _hook = [None]
    mod.set_axon_ntff_profile_hook = lambda h: _hook.__setitem__(0, h)
    mod.get_axon_ntff_profile_hook = lambda: _hook[0]
    sys.modules["antenv.axon_hooks"] = mod
    try:
        from trn_agent_boot.trn_boot import _ntff_profile_via_ctypes

        mod.set_axon_ntff_profile_hook(
            _ntff_profile_via_ctypes("/opt/axon/libaxon_pjrt.so")
        )
    except Exception:
        pass


def build():
    nc = bacc.Bacc("TRN2", target_bir_lowering=False, debug=False,
                   num_devices=NCORES)
    xT = nc.dram_tensor("xT", [C, N], BF16, kind="ExternalInput")
    wqk = nc.dram_tensor("wqkT", [C, DQK], BF16, kind="ExternalInput")
    wv = nc.dram_tensor("wvT", [C, DV], BF16, kind="ExternalInput")
    wp = nc.dram_tensor("wpT", [DV, C], BF16, kind="ExternalInput")
    mk = nc.dram_tensor("maskT", [N, N], BF16, kind="ExternalInput")
    yT = nc.dram_tensor("yT", [C, N], F32, kind="ExternalOutput")

    with tile.TileContext(nc) as tc, ExitStack() as ctx:
        consts = ctx.enter_context(tc.tile_pool(name="consts", bufs=1))
        xin = ctx.enter_context(tc.tile_pool(name="xin", bufs=16))
        ptp = ctx.enter_context(tc.tile_pool(name="ptp", bufs=4))
        pt2p = ctx.enter_context(tc.tile_pool(name="pt2p", bufs=24))
        ysb = ctx.enter_context(tc.tile_pool(name="ysb", bufs=2))
        dnp = ctx.enter_context(tc.tile_pool(name="dnp", bufs=2))
        rbp = ctx.enter_context(tc.tile_pool(name="rbp", bufs=2))
        # PSUM: 3x 2-bank + 2x 1-bank tiles = 8 banks static.
        pool2 = ctx.enter_context(tc.tile_pool(name="pool2", bufs=3, space="PSUM"))
        pool1 = ctx.enter_context(tc.tile_pool(name="pool1", bufs=2, space="PSUM"))

        # ---- resident weights (wp loaded late, after phase-A x traffic)
        wqk_sb = consts.tile([128, CK, DQK], BF16)
        wv_sb = consts.tile([128, CK, DV], BF16)
        wp_sb = consts.tile([128, 2, C], BF16)
        nc.sync.dma_start(out=wqk_sb,
                          in_=wqk[:].rearrange("(co ci) d -> ci co d", ci=128))

        # ---- per-chunk / per-mtile intermediates (fine-grained deps)
        qkT = [[consts.tile([128, NCK], BF16, name=f"qk_m{m}_c{c}")
                for c in range(NCH)] for m in range(4)]
        vb = [consts.tile([128, NH, HD + 1], BF16, name=f"vb_{mt}")
              for mt in range(MT)]
        mask_sb = [consts.tile([128, N], BF16, name=f"mask_m{mt}")
                   for mt in range(MT)]
        ot_sb = [consts.tile([128, 2, NCK], BF16, name=f"ot_n{ncb}")
                 for ncb in range(NCH)]

        # V_aug ones column (during DMA head; engines idle)
        for mt in range(MT):
            nc.gpsimd.memset(vb[mt][:, :, HD:HD + 1], 1.0)

        warm = consts.tile([128, NCK], BF16, name="warm")
        nc.vector.memset(warm[:, 0:NCK], 0.0)

        def emit_warmup(n):
            pwarm = pool1.tile([128, NCK], F32, tag="p1", name="pwarm")
            for i in range(n):
                nc.tensor.matmul(pwarm, lhsT=warm[:, 0:128], rhs=warm,
                                 start=True, stop=True)

        def emit_mask(mts):
            for mt in mts:
                nc.sync.dma_start(out=mask_sb[mt],
                                  in_=mk[mt * 128:(mt + 1) * 128, :])

        def emit_x_dma(c):
            xts = []
            for cc in range(CK):
                xt = xin.tile([128, NCK], BF16)
                nc.sync.dma_start(
                    out=xt[:, 0:NCK // 2],
                    in_=xT[cc * 128:(cc + 1) * 128,
                           c * NCK:c * NCK + NCK // 2])
                nc.sync.dma_start(
                    out=xt[:, NCK // 2:],
                    in_=xT[cc * 128:(cc + 1) * 128,
                           c * NCK + NCK // 2:(c + 1) * NCK])
                xts.append(xt)
            return xts

        # ---------------- phase A: q/k segments (paired so hp=0's Q+K
        # tiles land first), then v segments; PSUM->SBUF copies on
        # Scalar/DVE (early phase, exp stream is light)
        def emit_A_qk(c, xts, pair):
            ms = (0, 2) if pair == 0 else (1, 3)
            pa = pool2.tile([128, 2, NCK], F32, tag="p2")
            for cc in range(CK):
                for mm, m in enumerate(ms):
                    nc.tensor.matmul(
                        pa[:, mm, :],
                        lhsT=wqk_sb[:, cc, m * 128:(m + 1) * 128],
                        rhs=xts[cc], start=(cc == 0), stop=(cc == CK - 1))
            for mm, m in enumerate(ms):
                if mm == 0:
                    nc.scalar.copy(out=qkT[m][c], in_=pa[:, mm, :])
                else:
                    nc.vector.tensor_copy(out=qkT[m][c], in_=pa[:, mm, :])

        def emit_A_v(c, xts, seg):
            pv = pool2.tile([128, 2, NCK], F32, tag="p2")
            for cc in range(CK):
                for jj in range(2):
                    j = 2 * seg + jj
                    nc.tensor.matmul(
                        pv[:, jj, 0:DV],
                        lhsT=xts[cc][:, j * 128:(j + 1) * 128],
                        rhs=wv_sb[:, cc, :],
                        start=(cc == 0), stop=(cc == CK - 1))
            for jj in range(2):
                mt = 4 * c + 2 * seg + jj
                src = pv[:, jj, 0:DV].rearrange("p (h d) -> p h d", h=NH)
                if jj == 0:
                    nc.scalar.copy(out=vb[mt][:, :, 0:HD], in_=src)
                else:
                    nc.vector.tensor_copy(out=vb[mt][:, :, 0:HD], in_=src)

        # ---------------- attention block (hp, ncb)
        class Block:
            def __init__(self, hp, ncb):
                self.hp, self.ncb = hp, ncb
                self.nsl = slice(ncb * NCK, (ncb + 1) * NCK)
                self.pso = None
                self.pts = {}     # mt -> masked-P tile awaiting PV
                self.next_pv = 0
                self.next_step = 0

            def step1(self):
                if self.next_step < MT:
                    self.steps([self.next_step])

            def steps(self, mts):
                mq, mkt = self.hp, 2 + self.hp
                for mt in mts:
                    self.next_step = mt + 1
                    pss = pool2.tile([128, 2, NCK], F32, tag="p2")
                    for par in range(2):
                        po = par * 64
                        nc.tensor.matmul(
                            pss[:, par, :],
                            lhsT=qkT[mkt][mt // 4][po:po + 64,
                                                   (mt % 4) * 128:
                                                   (mt % 4 + 1) * 128],
                            rhs=qkT[mq][self.ncb][po:po + 64, :],
                            start=True, stop=True)
                    pt = ptp.tile([128, 2, NCK], BF16)
                    nc.scalar.activation(
                        out=pt, in_=pss,
                        func=mybir.ActivationFunctionType.Exp, scale=SCALE)
                    pt2 = pt2p.tile([128, 2, NCK], BF16)
                    for par in range(2):
                        nc.vector.tensor_mul(out=pt2[:, par, :],
                                             in0=pt[:, par, :],
                                             in1=mask_sb[mt][:, self.nsl])
                    self.pts[mt] = pt2

            def pvs(self, n):
                if self.pso is None:
                    self.pso = [pool1.tile([128, NCK], F32, tag="p1",
                                           name=f"pso{self.hp}_{self.ncb}_{p}")
                                for p in range(2)]
                for _ in range(n):
                    mt = self.next_pv
                    pt2 = self.pts.pop(mt)
                    for par in range(2):
                        nc.tensor.matmul(
                            self.pso[par][0:HD + 1, :],
                            lhsT=vb[mt][:, 2 * self.hp + par, :],
                            rhs=pt2[:, par, :],
                            start=(mt == 0), stop=(mt == MT - 1))
                    self.next_pv += 1

            def finalize(self):
                for par in range(2):
                    po = par * 64
                    den = dnp.tile([1, NCK], F32, tag="den")
                    nc.vector.tensor_copy(out=den,
                                          in_=self.pso[par][HD:HD + 1, :])
                    rec = dnp.tile([1, NCK], F32, tag="rec")
                    nc.vector.reciprocal_approx_fast(out=rec, in_=den)
                    rb = rbp.tile([64, NCK], F32)
                    nc.gpsimd.partition_broadcast(rb, rec)
                    nc.vector.tensor_mul(
                        out=ot_sb[self.ncb][po:po + 64, self.hp, :],
                        in0=self.pso[par][0:HD, :], in1=rb)

        # ---------------- output projection piece: 2 et rows per call
        def emit_proj_piece(ncb, pair):
            nsl = slice(ncb * NCK, (ncb + 1) * NCK)
            psy = pool2.tile([128, 2, NCK], F32, tag="p2")
            for ee in range(2):
                et = 2 * pair + ee
                for dk in range(2):
                    nc.tensor.matmul(
                        psy[:, ee, :],
                        lhsT=wp_sb[:, dk, et * 128:(et + 1) * 128],
                        rhs=ot_sb[ncb][:, dk, :],
                        start=(dk == 0), stop=(dk == 1))
            yt = ysb.tile([128, 2, NCK], F32)
            nc.vector.tensor_copy(out=yt[:, 0, :], in_=psy[:, 0, :])
            nc.vector.tensor_copy(out=yt[:, 1, :], in_=psy[:, 1, :])
            for ee in range(2):
                et = 2 * pair + ee
                nc.sync.dma_start(out=yT[et * 128:(et + 1) * 128, nsl],
                                  in_=yt[:, ee, :])

        # ---------------- schedule: one block step per phase-A segment
        # (keeps each score's exp dependency a full segment away -> the PE
        # never waits mid-phase-A and holds its high p-state)
        b0 = Block(0, 0)
        b1 = Block(1, 0)
        b01 = Block(0, 1)
        xts = emit_x_dma(0)
        emit_mask([0, 1])
        nc.sync.dma_start(out=wv_sb,
                          in_=wv[:].rearrange("(co ci) d -> ci co d", ci=128))
        emit_warmup(24)
        emit_mask([2, 3])
        emit_A_qk(0, xts, 0)
        b0.step1()
        emit_A_qk(0, xts, 1)
        b0.step1()
        emit_A_v(0, xts, 0)
        b0.step1()
        emit_A_v(0, xts, 1)
        nxts = emit_x_dma(1)
        emit_mask([4, 5, 6, 7])
        xts = nxts
        emit_A_qk(1, xts, 0)
        b0.step1()
        emit_A_qk(1, xts, 1)
        b0.step1()
        emit_A_v(1, xts, 0)
        b0.pvs(2)
        b0.step1()
        emit_A_v(1, xts, 1)
        b1.step1()
        nxts = emit_x_dma(2)
        emit_mask([8, 9, 10, 11])
        xts = nxts
        emit_A_qk(2, xts, 0)
        b0.step1()
        emit_A_qk(2, xts, 1)
        b0.step1()
        b0.pvs(2)
        emit_A_v(2, xts, 0)
        b0.step1()
        emit_A_v(2, xts, 1)
        b1.step1()
        b1.step1()
        b01.step1()
        nxts = emit_x_dma(3)
        emit_mask([12, 13, 14, 15])
        xts = nxts
        emit_A_qk(3, xts, 0)
        b0.step1()
        emit_A_qk(3, xts, 1)
        b0.step1()
        b0.pvs(2)
        emit_A_v(3, xts, 0)
        b0.step1()
        emit_A_v(3, xts, 1)
        b1.step1()
        b0.pvs(2)
        nc.sync.dma_start(out=wp_sb,
                          in_=wp[:].rearrange("(dk ci) e -> ci dk e", ci=128))

        # post-A steady pipeline: step-interleaved boundaries + proj fills
        order = [(1, 0), (0, 1), (1, 1), (0, 2), (1, 2), (0, 3), (1, 3)]
        blocks = {(1, 0): b1, (0, 1): b01}
        prev = b0
        pending_proj = []
        for hp, ncb in order:
            blk = blocks.get((hp, ncb)) or Block(hp, ncb)
            # boundary: prev's last steps alternate with blk's next steps;
            # prev's remaining PVs drain 2 per slot
            for i in range(4):
                prev.step1()
                blk.step1()
                prev.pvs(2)
            # finalize is delayed 3 steps: its DVE ops (normalize TTs
            # waiting on the gpsimd broadcasts) would otherwise head-block
            # the new block's mask stream at the boundary
            for i in range(8):
                blk.step1()
                if i == 2:
                    prev.finalize()
                    if prev.hp == 1:
                        pending_proj += [(prev.ncb, pair)
                                         for pair in range(4)]
                if pending_proj and i in (3, 5, 7):
                    emit_proj_piece(*pending_proj.pop(0))
                if i >= 3:
                    blk.pvs(2 if i <= 5 else 1)
            prev = blk
        # tail: last block steps 12..15, drain, finalize, proj(3)
        for i in range(4):
            prev.step1()
            prev.pvs(2)
        twarm = pool2.tile([128, 2, NCK], F32, tag="p2")
        for i in range(8):
            nc.tensor.matmul(twarm[:, 0, :], lhsT=warm[:, 0:128], rhs=warm,
                             start=True, stop=True)
        prev.finalize()
        for pair in range(4):
            emit_proj_piece(prev.ncb, pair)

    nc.compile()
    return nc


_NC = None


def _get_nc():
    global _NC
    if _NC is None:
        _NC = build()
    return _NC


def make_in_maps(x, mask, W_qkv, W_proj):
    x = np.asarray(x, dtype=np.float32)
    mask = np.asarray(mask)
    W_qkv = np.asarray(W_qkv, dtype=np.float32)
    W_proj = np.asarray(W_proj, dtype=np.float32)
    bf = ml_dtypes.bfloat16
    in_maps = []
    for cid in range(NCORES):
        b, hg = divmod(cid, 4)
        rs = slice(hg * 256, (hg + 1) * 256)
        wq = W_qkv[0 * C:1 * C][rs]          # [256, 1024]
        wk = W_qkv[1 * C:2 * C][rs]
        wvs = W_qkv[2 * C:3 * C][rs]
        in_maps.append({
            "xT": np.ascontiguousarray(x[b].T).astype(bf),
            "wqkT": np.ascontiguousarray(
                np.concatenate([wq, wk], axis=0).T).astype(bf),
            "wvT": np.ascontiguousarray(wvs.T).astype(bf),
            "wpT": np.ascontiguousarray(W_proj[:, rs].T).astype(bf),
            "maskT": np.ascontiguousarray(mask[b, 0].T).astype(bf),
        })
    return in_maps


LAST_EXEC_NS = None
LAST_MEAN_EXEC_NS = None


def kernel(x, mask, W_qkv, W_proj, b_proj):
    global LAST_EXEC_NS, LAST_MEAN_EXEC_NS
    trace = bool(int(os.environ.get("TRNK_TRACE", "0")))
    if trace:
        _ensure_ntff_hook()
    nc = _get_nc()
    in_maps = make_in_maps(x, mask, W_qkv, W_proj)
    res = run_bass_kernel_spmd(nc, in_maps, list(range(NCORES)), trace=trace)
    LAST_EXEC_NS = res.exec_time_ns
    LAST_MEAN_EXEC_NS = res.mean_exec_time_ns
    y = np.zeros((2, N, C), dtype=np.float32)
    for cid in range(NCORES):
        b = cid // 4
        y[b] += np.asarray(res.results[cid]["yT"], dtype=np.float32).T
    y += np.asarray(b_proj, dtype=np.float32)[None, None, :]
    return y


# revision 29
# speedup vs baseline: 1.0152x; 1.0022x over previous
"""Multi-head attention (B=2, N=2048, C=1024, H=16) on 8 Trainium2 cores.

Sharding: core cid = (b, hg) with b = cid//4, hg = cid%4.  Data-parallel on
batch, 4-way tensor-parallel on heads (4 heads / 256 dims per core).  Each
core computes q/k/v projections for its head slice, full (masked-softmax)
attention for its 4 heads, and a partial output projection y^T = Wp_slice^T
-contracted over its 256 dims.  Host sums the 4 partials per batch and adds
the proj bias.

v4 pipeline:
  - Everything bf16 on-device (f32 PSUM accumulation); host pre-casts x, W,
    and the mask (no on-device uint8->bf16 casts).
  - Block (hp, ncb) = 16-m-tile sweep: scores S^T = K^T_h x Q^T_h (head
    pair packed at partition offsets 0/64), exp on Scalar (scale folded),
    mask multiply on DVE into a fresh tile, PV lag-4 behind scores.
  - Step-granular boundary interleave: the last 4 steps of block P
    alternate with the first 4 steps of block B, so the Scalar engine's
    exp stream never drains at block handoffs; P's PV flush + normalize
    land behind B's early steps as PE fill.
  - proj-C psy pairs (2-bank PSUM tiles) spread through the following
    block's steps; psy copy-out split Scalar/DVE.
  - PSUM: pool2 3x2-bank (phase-A segments / score ping-pong / psy pairs)
    + pool1 2x1-bank (O^T accumulators) = 8 banks.
"""

import os
import sys
import types
from contextlib import ExitStack

import numpy as np
import ml_dtypes

import concourse.bass as bass
import concourse.mybir as mybir
import concourse.tile as tile
from concourse import bacc
from concourse.bass_utils import run_bass_kernel_spmd

# ---------------------------------------------------------------- constants
N = 2048          # sequence length
C = 1024          # model dim
NH = 4            # heads per core
HD = 64           # head dim
DQK = 2 * NH * HD # 512: q rows then k rows in qk^T
DV = NH * HD      # 256
NCK = 512         # n-chunk size
NCH = N // NCK    # 4 n-chunks
MT = N // 128     # 16 m-tiles
CK = C // 128     # 8 contraction chunks
SCALE = HD ** -0.5
NCORES = 8

F32 = mybir.dt.float32
BF16 = mybir.dt.bfloat16


def _ensure_ntff_hook():
    """bass_utils' trace path imports antenv.axon_hooks, which this image
    lacks; inject it and register the ctypes-based NTFF profile hook."""
    if "antenv.axon_hooks" in sys.modules:
        return
    mod = types.ModuleType("antenv.axon_hooks")
    _hook = [None]
    mod.set_axon_ntff_profile_hook = lambda h: _hook.__setitem__(0, h)
    mod.get_axon_ntff_profile_hook = lambda: _hook[0]
    sys.modules["antenv.axon_hooks"] = mod
    try:
        from trn_agent_boot.trn_boot import _ntff_profile_via_ctypes

        mod.set_axon_ntff_profile_hook(
            _ntff_profile_via_ctypes("/opt/axon/libaxon_pjrt.so")
        )
    except Exception:
        pass


def build():
    nc = bacc.Bacc("TRN2", target_bir_lowering=False, debug=False,
                   num_devices=NCORES)
    xT = nc.dram_tensor("xT", [C, N], BF16, kind="ExternalInput")
    wqk = nc.dram_tensor("wqkT", [C, DQK], BF16, kind="ExternalInput")
    wv = nc.dram_tensor("wvT", [C, DV], BF16, kind="ExternalInput")
    wp = nc.dram_tensor("wpT", [DV, C], BF16, kind="ExternalInput")
    mk = nc.dram_tensor("maskT", [N, N], BF16, kind="ExternalInput")
    yT = nc.dram_tensor("yT", [C, N], F32, kind="ExternalOutput")

    with tile.TileContext(nc) as tc, ExitStack() as ctx:
        consts = ctx.enter_context(tc.tile_pool(name="consts", bufs=1))
        xin = ctx.enter_context(tc.tile_pool(name="xin", bufs=16))
        ptp = ctx.enter_context(tc.tile_pool(name="ptp", bufs=4))
        pt2p = ctx.enter_context(tc.tile_pool(name="pt2p", bufs=24))
        ysb = ctx.enter_context(tc.tile_pool(name="ysb", bufs=2))
        dnp = ctx.enter_context(tc.tile_pool(name="dnp", bufs=2))
        rbp = ctx.enter_context(tc.tile_pool(name="rbp", bufs=2))
        # PSUM: 3x 2-bank + 2x 1-bank tiles = 8 banks static.
        pool2 = ctx.enter_context(tc.tile_pool(name="pool2", bufs=3, space="PSUM"))
        pool1 = ctx.enter_context(tc.tile_pool(name="pool1", bufs=2, space="PSUM"))

        # ---- resident weights (wp loaded late, after phase-A x traffic)
        wqk_sb = consts.tile([128, CK, DQK], BF16)
        wv_sb = consts.tile([128, CK, DV], BF16)
        wp_sb = consts.tile([128, 2, C], BF16)
        nc.sync.dma_start(out=wqk_sb,
                          in_=wqk[:].rearrange("(co ci) d -> ci co d", ci=128))

        # ---- per-chunk / per-mtile intermediates (fine-grained deps)
        qkT = [[consts.tile([128, NCK], BF16, name=f"qk_m{m}_c{c}")
                for c in range(NCH)] for m in range(4)]
        vb = [consts.tile([128, NH, HD + 1], BF16, name=f"vb_{mt}")
              for mt in range(MT)]
        mask_sb = [consts.tile([128, N], BF16, name=f"mask_m{mt}")
                   for mt in range(MT)]
        ot_sb = [consts.tile([128, 2, NCK], BF16, name=f"ot_n{ncb}")
                 for ncb in range(NCH)]

        # V_aug ones column (during DMA head; engines idle)
        for mt in range(MT):
            nc.gpsimd.memset(vb[mt][:, :, HD:HD + 1], 1.0)

        warm = consts.tile([128, NCK], BF16, name="warm")
        nc.vector.memset(warm[:, 0:NCK], 0.0)

        def emit_warmup(n):
            pwarm = pool1.tile([128, NCK], F32, tag="p1", name="pwarm")
            for i in range(n):
                nc.tensor.matmul(pwarm, lhsT=warm[:, 0:128], rhs=warm,
                                 start=True, stop=True)

        def emit_mask(mts):
            for mt in mts:
                nc.sync.dma_start(out=mask_sb[mt],
                                  in_=mk[mt * 128:(mt + 1) * 128, :])

        def emit_x_dma(c):
            xts = []
            for cc in range(CK):
                xt = xin.tile([128, NCK], BF16)
                nc.sync.dma_start(
                    out=xt[:, 0:NCK // 2],
                    in_=xT[cc * 128:(cc + 1) * 128,
                           c * NCK:c * NCK + NCK // 2])
                nc.sync.dma_start(
                    out=xt[:, NCK // 2:],
                    in_=xT[cc * 128:(cc + 1) * 128,
                           c * NCK + NCK // 2:(c + 1) * NCK])
                xts.append(xt)
            return xts

        # ---------------- phase A: q/k segments (paired so hp=0's Q+K
        # tiles land first), then v segments; PSUM->SBUF copies on
        # Scalar/DVE (early phase, exp stream is light)
        def emit_A_qk(c, xts, pair):
            ms = (0, 2) if pair == 0 else (1, 3)
            pa = pool2.tile([128, 2, NCK], F32, tag="p2")
            for cc in range(CK):
                for mm, m in enumerate(ms):
                    nc.tensor.matmul(
                        pa[:, mm, :],
                        lhsT=wqk_sb[:, cc, m * 128:(m + 1) * 128],
                        rhs=xts[cc], start=(cc == 0), stop=(cc == CK - 1))
            for mm, m in enumerate(ms):
                if mm == 0:
                    nc.scalar.copy(out=qkT[m][c], in_=pa[:, mm, :])
                else:
                    nc.vector.tensor_copy(out=qkT[m][c], in_=pa[:, mm, :])

        def emit_A_v(c, xts, seg):
            pv = pool2.tile([128, 2, NCK], F32, tag="p2")
            for cc in range(CK):
                for jj in range(2):
                    j = 2 * seg + jj
                    nc.tensor.matmul(
                        pv[:, jj, 0:DV],
                        lhsT=xts[cc][:, j * 128:(j + 1) * 128],
                        rhs=wv_sb[:, cc, :],
                        start=(cc == 0), stop=(cc == CK - 1))
            for jj in range(2):
                mt = 4 * c + 2 * seg + jj
                src = pv[:, jj, 0:DV].rearrange("p (h d) -> p h d", h=NH)
                if jj == 0:
                    nc.scalar.copy(out=vb[mt][:, :, 0:HD], in_=src)
                else:
                    nc.vector.tensor_copy(out=vb[mt][:, :, 0:HD], in_=src)

        # ---------------- attention block (hp, ncb)
        class Block:
            def __init__(self, hp, ncb):
                self.hp, self.ncb = hp, ncb
                self.nsl = slice(ncb * NCK, (ncb + 1) * NCK)
                self.pso = None
                self.pts = {}     # mt -> masked-P tile awaiting PV
                self.next_pv = 0
                self.next_step = 0

            def step1(self):
                if self.next_step < MT:
                    self.steps([self.next_step])

            def steps(self, mts):
                mq, mkt = self.hp, 2 + self.hp
                for mt in mts:
                    self.next_step = mt + 1
                    pss = pool2.tile([128, 2, NCK], F32, tag="p2")
                    for par in range(2):
                        po = par * 64
                        nc.tensor.matmul(
                            pss[:, par, :],
                            lhsT=qkT[mkt][mt // 4][po:po + 64,
                                                   (mt % 4) * 128:
                                                   (mt % 4 + 1) * 128],
                            rhs=qkT[mq][self.ncb][po:po + 64, :],
                            start=True, stop=True)
                    pt = ptp.tile([128, 2, NCK], BF16)
                    nc.scalar.activation(
                        out=pt, in_=pss,
                        func=mybir.ActivationFunctionType.Exp, scale=SCALE)
                    pt2 = pt2p.tile([128, 2, NCK], BF16)
                    for par in range(2):
                        nc.vector.tensor_mul(out=pt2[:, par, :],
                                             in0=pt[:, par, :],
                                             in1=mask_sb[mt][:, self.nsl])
                    self.pts[mt] = pt2

            def pvs(self, n):
                if self.pso is None:
                    self.pso = [pool1.tile([128, NCK], F32, tag="p1",
                                           name=f"pso{self.hp}_{self.ncb}_{p}")
                                for p in range(2)]
                for _ in range(n):
                    mt = self.next_pv
                    pt2 = self.pts.pop(mt)
                    for par in range(2):
                        nc.tensor.matmul(
                            self.pso[par][0:HD + 1, :],
                            lhsT=vb[mt][:, 2 * self.hp + par, :],
                            rhs=pt2[:, par, :],
                            start=(mt == 0), stop=(mt == MT - 1))
                    self.next_pv += 1

            def finalize(self):
                for par in range(2):
                    po = par * 64
                    den = dnp.tile([1, NCK], F32, tag="den")
                    nc.vector.tensor_copy(out=den,
                                          in_=self.pso[par][HD:HD + 1, :])
                    rec = dnp.tile([1, NCK], F32, tag="rec")
                    nc.vector.reciprocal_approx_fast(out=rec, in_=den)
                    rb = rbp.tile([64, NCK], F32)
                    nc.gpsimd.partition_broadcast(rb, rec)
                    nc.vector.tensor_mul(
                        out=ot_sb[self.ncb][po:po + 64, self.hp, :],
                        in0=self.pso[par][0:HD, :], in1=rb)

        # ---------------- output projection piece: 2 et rows per call
        def emit_proj_piece(ncb, pair, tail=False):
            nsl = slice(ncb * NCK, (ncb + 1) * NCK)
            psy = pool2.tile([128, 2, NCK], F32, tag="p2")
            for ee in range(2):
                et = 2 * pair + ee
                for dk in range(2):
                    nc.tensor.matmul(
                        psy[:, ee, :],
                        lhsT=wp_sb[:, dk, et * 128:(et + 1) * 128],
                        rhs=ot_sb[ncb][:, dk, :],
                        start=(dk == 0), stop=(dk == 1))
            yt = ysb.tile([128, 2, NCK], F32)
            nc.vector.tensor_copy(out=yt[:, 0, :], in_=psy[:, 0, :])
            if tail:
                nc.scalar.copy(out=yt[:, 1, :], in_=psy[:, 1, :])
            else:
                nc.vector.tensor_copy(out=yt[:, 1, :], in_=psy[:, 1, :])
            for ee in range(2):
                et = 2 * pair + ee
                nc.sync.dma_start(out=yT[et * 128:(et + 1) * 128, nsl],
                                  in_=yt[:, ee, :])

        # ---------------- schedule: one block step per phase-A segment
        # (keeps each score's exp dependency a full segment away -> the PE
        # never waits mid-phase-A and holds its high p-state)
        b0 = Block(0, 0)
        b1 = Block(1, 0)
        b01 = Block(0, 1)
        xts = emit_x_dma(0)
        emit_mask([0, 1])
        nc.sync.dma_start(out=wv_sb,
                          in_=wv[:].rearrange("(co ci) d -> ci co d", ci=128))
        emit_warmup(24)
        emit_mask([2, 3])
        emit_A_qk(0, xts, 0)
        b0.step1()
        emit_A_qk(0, xts, 1)
        b0.step1()
        emit_A_v(0, xts, 0)
        b0.step1()
        emit_A_v(0, xts, 1)
        b1.step1()
        nxts = emit_x_dma(1)
        emit_mask([4, 5, 6, 7])
        xts = nxts
        emit_A_qk(1, xts, 0)
        b0.step1()
        emit_A_qk(1, xts, 1)
        b0.step1()
        emit_A_v(1, xts, 0)
        b0.pvs(2)
        b0.step1()
        emit_A_v(1, xts, 1)
        b1.step1()
        nxts = emit_x_dma(2)
        emit_mask([8, 9, 10, 11])
        xts = nxts
        emit_A_qk(2, xts, 0)
        b0.step1()
        emit_A_qk(2, xts, 1)
        b0.step1()
        b0.pvs(2)
        emit_A_v(2, xts, 0)
        b0.step1()
        emit_A_v(2, xts, 1)
        b01.step1()
        nxts = emit_x_dma(3)
        emit_mask([12, 13, 14, 15])
        xts = nxts
        emit_A_qk(3, xts, 0)
        b0.step1()
        emit_A_qk(3, xts, 1)
        b0.step1()
        b0.pvs(2)
        emit_A_v(3, xts, 0)
        b0.step1()
        emit_A_v(3, xts, 1)
        b1.step1()
        b0.pvs(2)
        nc.sync.dma_start(out=wp_sb,
                          in_=wp[:].rearrange("(dk ci) e -> ci dk e", ci=128))

        # post-A steady pipeline: step-interleaved boundaries + proj fills
        order = [(1, 0), (0, 1), (1, 1), (0, 2), (1, 2), (0, 3), (1, 3)]
        blocks = {(1, 0): b1, (0, 1): b01}
        prev = b0
        pending_proj = []
        for hp, ncb in order:
            blk = blocks.get((hp, ncb)) or Block(hp, ncb)
            # boundary: prev's last steps alternate with blk's next steps;
            # prev's remaining PVs drain 2 per slot
            for i in range(4):
                prev.step1()
                blk.step1()
                prev.pvs(2)
            # finalize is delayed 3 steps: its DVE ops (normalize TTs
            # waiting on the gpsimd broadcasts) would otherwise head-block
            # the new block's mask stream at the boundary
            for i in range(8):
                blk.step1()
                if i == 2:
                    prev.finalize()
                    if prev.hp == 1:
                        pending_proj += [(prev.ncb, pair)
                                         for pair in range(4)]
                if pending_proj and i in (3, 5, 7):
                    emit_proj_piece(*pending_proj.pop(0))
                if i >= 3:
                    blk.pvs(2 if i <= 5 else 1)
            prev = blk
        # tail: last block steps 12..15, drain, finalize, proj(3)
        for i in range(4):
            prev.step1()
            prev.pvs(2)
        twarm = pool2.tile([128, 2, NCK], F32, tag="p2")
        for i in range(8):
            nc.tensor.matmul(twarm[:, 0, :], lhsT=warm[:, 0:128], rhs=warm,
                             start=True, stop=True)
        prev.finalize()
        for pair in range(4):
            emit_proj_piece(prev.ncb, pair, tail=True)

    nc.compile()
    return nc


_NC = None


def _get_nc():
    global _NC
    if _NC is None:
        _NC = build()
    return _NC


def make_in_maps(x, mask, W_qkv, W_proj):
    x = np.asarray(x, dtype=np.float32)
    mask = np.asarray(mask)
    W_qkv = np.asarray(W_qkv, dtype=np.float32)
    W_proj = np.asarray(W_proj, dtype=np.float32)
    bf = ml_dtypes.bfloat16
    in_maps = []
    for cid in range(NCORES):
        b, hg = divmod(cid, 4)
        rs = slice(hg * 256, (hg + 1) * 256)
        wq = W_qkv[0 * C:1 * C][rs]          # [256, 1024]
        wk = W_qkv[1 * C:2 * C][rs]
        wvs = W_qkv[2 * C:3 * C][rs]
        in_maps.append({
            "xT": np.ascontiguousarray(x[b].T).astype(bf),
            "wqkT": np.ascontiguousarray(
                np.concatenate([wq, wk], axis=0).T).astype(bf),
            "wvT": np.ascontiguousarray(wvs.T).astype(bf),
            "wpT": np.ascontiguousarray(W_proj[:, rs].T).astype(bf),
            "maskT": np.ascontiguousarray(mask[b, 0].T).astype(bf),
        })
    return in_maps


LAST_EXEC_NS = None
LAST_MEAN_EXEC_NS = None


def kernel(x, mask, W_qkv, W_proj, b_proj):
    global LAST_EXEC_NS, LAST_MEAN_EXEC_NS
    trace = bool(int(os.environ.get("TRNK_TRACE", "0")))
    if trace:
        _ensure_ntff_hook()
    nc = _get_nc()
    in_maps = make_in_maps(x, mask, W_qkv, W_proj)
    res = run_bass_kernel_spmd(nc, in_maps, list(range(NCORES)), trace=trace)
    LAST_EXEC_NS = res.exec_time_ns
    LAST_MEAN_EXEC_NS = res.mean_exec_time_ns
    y = np.zeros((2, N, C), dtype=np.float32)
    for cid in range(NCORES):
        b = cid // 4
        y[b] += np.asarray(res.results[cid]["yT"], dtype=np.float32).T
    y += np.asarray(b_proj, dtype=np.float32)[None, None, :]
    return y


# revision 30
# speedup vs baseline: 1.0234x; 1.0080x over previous
"""Multi-head attention (B=2, N=2048, C=1024, H=16) on 8 Trainium2 cores.

Sharding: core cid = (b, hg) with b = cid//4, hg = cid%4.  Data-parallel on
batch, 4-way tensor-parallel on heads (4 heads / 256 dims per core).  Each
core computes q/k/v projections for its head slice, full (masked-softmax)
attention for its 4 heads, and a partial output projection y^T = Wp_slice^T
-contracted over its 256 dims.  Host sums the 4 partials per batch and adds
the proj bias.

v4 pipeline:
  - Everything bf16 on-device (f32 PSUM accumulation); host pre-casts x, W,
    and the mask (no on-device uint8->bf16 casts).
  - Block (hp, ncb) = 16-m-tile sweep: scores S^T = K^T_h x Q^T_h (head
    pair packed at partition offsets 0/64), exp on Scalar (scale folded),
    mask multiply on DVE into a fresh tile, PV lag-4 behind scores.
  - Step-granular boundary interleave: the last 4 steps of block P
    alternate with the first 4 steps of block B, so the Scalar engine's
    exp stream never drains at block handoffs; P's PV flush + normalize
    land behind B's early steps as PE fill.
  - proj-C psy pairs (2-bank PSUM tiles) spread through the following
    block's steps; psy copy-out split Scalar/DVE.
  - PSUM: pool2 3x2-bank (phase-A segments / score ping-pong / psy pairs)
    + pool1 2x1-bank (O^T accumulators) = 8 banks.
"""

import os
import sys
import types
from contextlib import ExitStack

import numpy as np
import ml_dtypes

import concourse.bass as bass
import concourse.mybir as mybir
import concourse.tile as tile
from concourse import bacc
from concourse.bass_utils import run_bass_kernel_spmd

# ---------------------------------------------------------------- constants
N = 2048          # sequence length
C = 1024          # model dim
NH = 4            # heads per core
HD = 64           # head dim
DQK = 2 * NH * HD # 512: q rows then k rows in qk^T
DV = NH * HD      # 256
NCK = 512         # n-chunk size
NCH = N // NCK    # 4 n-chunks
MT = N // 128     # 16 m-tiles
CK = C // 128     # 8 contraction chunks
SCALE = HD ** -0.5
NCORES = 8

F32 = mybir.dt.float32
BF16 = mybir.dt.bfloat16


def _ensure_ntff_hook():
    """bass_utils' trace path imports antenv.axon_hooks, which this image
    lacks; inject it and register the ctypes-based NTFF profile hook."""
    if "antenv.axon_hooks" in sys.modules:
        return
    mod = types.ModuleType("antenv.axon_hooks")
    _hook = [None]
    mod.set_axon_ntff_profile_hook = lambda h: _hook.__setitem__(0, h)
    mod.get_axon_ntff_profile_hook = lambda: _hook[0]
    sys.modules["antenv.axon_hooks"] = mod
    try:
        from trn_agent_boot.trn_boot import _ntff_profile_via_ctypes

        mod.set_axon_ntff_profile_hook(
            _ntff_profile_via_ctypes("/opt/axon/libaxon_pjrt.so")
        )
    except Exception:
        pass


def build():
    nc = bacc.Bacc("TRN2", target_bir_lowering=False, debug=False,
                   num_devices=NCORES)
    xT = nc.dram_tensor("xT", [C, N], BF16, kind="ExternalInput")
    wqk = nc.dram_tensor("wqkT", [C, DQK], BF16, kind="ExternalInput")
    wv = nc.dram_tensor("wvT", [C, DV], BF16, kind="ExternalInput")
    wp = nc.dram_tensor("wpT", [DV, C], BF16, kind="ExternalInput")
    mk = nc.dram_tensor("maskT", [N, N], BF16, kind="ExternalInput")
    yT = nc.dram_tensor("yT", [C, N], F32, kind="ExternalOutput")

    with tile.TileContext(nc) as tc, ExitStack() as ctx:
        consts = ctx.enter_context(tc.tile_pool(name="consts", bufs=1))
        xin = ctx.enter_context(tc.tile_pool(name="xin", bufs=16))
        ptp = ctx.enter_context(tc.tile_pool(name="ptp", bufs=4))
        pt2p = ctx.enter_context(tc.tile_pool(name="pt2p", bufs=24))
        ysb = ctx.enter_context(tc.tile_pool(name="ysb", bufs=2))
        dnp = ctx.enter_context(tc.tile_pool(name="dnp", bufs=2))
        rbp = ctx.enter_context(tc.tile_pool(name="rbp", bufs=2))
        # PSUM: 3x 2-bank + 2x 1-bank tiles = 8 banks static.
        pool2 = ctx.enter_context(tc.tile_pool(name="pool2", bufs=3, space="PSUM"))
        pool1 = ctx.enter_context(tc.tile_pool(name="pool1", bufs=2, space="PSUM"))

        # ---- resident weights (wp loaded late, after phase-A x traffic)
        wqk_sb = consts.tile([128, CK, DQK], BF16)
        wv_sb = consts.tile([128, CK, DV], BF16)
        wp_sb = consts.tile([128, 2, C], BF16)
        nc.sync.dma_start(out=wqk_sb,
                          in_=wqk[:].rearrange("(co ci) d -> ci co d", ci=128))

        # ---- per-chunk / per-mtile intermediates (fine-grained deps)
        qkT = [[consts.tile([128, NCK], BF16, name=f"qk_m{m}_c{c}")
                for c in range(NCH)] for m in range(4)]
        vb = [consts.tile([128, NH, HD + 1], BF16, name=f"vb_{mt}")
              for mt in range(MT)]
        mask_sb = [consts.tile([128, N], BF16, name=f"mask_m{mt}")
                   for mt in range(MT)]
        ot_sb = [consts.tile([128, 2, NCK], BF16, name=f"ot_n{ncb}")
                 for ncb in range(NCH)]

        # V_aug ones column (during DMA head; engines idle)
        for mt in range(MT):
            nc.gpsimd.memset(vb[mt][:, :, HD:HD + 1], 1.0)

        warm = consts.tile([128, NCK], BF16, name="warm")
        nc.vector.memset(warm[:, 0:NCK], 0.0)

        def emit_warmup(n):
            pwarm = pool1.tile([128, NCK], F32, tag="p1", name="pwarm")
            for i in range(n):
                nc.tensor.matmul(pwarm, lhsT=warm[:, 0:128], rhs=warm,
                                 start=True, stop=True)

        def emit_mask(mts):
            for mt in mts:
                nc.sync.dma_start(out=mask_sb[mt],
                                  in_=mk[mt * 128:(mt + 1) * 128, :])

        def emit_x_dma(c):
            xts = []
            for cc in range(CK):
                xt = xin.tile([128, NCK], BF16)
                nc.sync.dma_start(
                    out=xt[:, 0:NCK // 2],
                    in_=xT[cc * 128:(cc + 1) * 128,
                           c * NCK:c * NCK + NCK // 2])
                nc.sync.dma_start(
                    out=xt[:, NCK // 2:],
                    in_=xT[cc * 128:(cc + 1) * 128,
                           c * NCK + NCK // 2:(c + 1) * NCK])
                xts.append(xt)
            return xts

        # ---------------- phase A: q/k segments (paired so hp=0's Q+K
        # tiles land first), then v segments; PSUM->SBUF copies on
        # Scalar/DVE (early phase, exp stream is light)
        def emit_A_qk(c, xts, pair):
            ms = (0, 2) if pair == 0 else (1, 3)
            pa = pool2.tile([128, 2, NCK], F32, tag="p2")
            for cc in range(CK):
                for mm, m in enumerate(ms):
                    nc.tensor.matmul(
                        pa[:, mm, :],
                        lhsT=wqk_sb[:, cc, m * 128:(m + 1) * 128],
                        rhs=xts[cc], start=(cc == 0), stop=(cc == CK - 1))
            for mm, m in enumerate(ms):
                if mm == 0:
                    nc.scalar.copy(out=qkT[m][c], in_=pa[:, mm, :])
                else:
                    nc.vector.tensor_copy(out=qkT[m][c], in_=pa[:, mm, :])

        def emit_A_v(c, xts, seg):
            pv = pool2.tile([128, 2, NCK], F32, tag="p2")
            for cc in range(CK):
                for jj in range(2):
                    j = 2 * seg + jj
                    nc.tensor.matmul(
                        pv[:, jj, 0:DV],
                        lhsT=xts[cc][:, j * 128:(j + 1) * 128],
                        rhs=wv_sb[:, cc, :],
                        start=(cc == 0), stop=(cc == CK - 1))
            for jj in range(2):
                mt = 4 * c + 2 * seg + jj
                src = pv[:, jj, 0:DV].rearrange("p (h d) -> p h d", h=NH)
                if jj == 0:
                    nc.scalar.copy(out=vb[mt][:, :, 0:HD], in_=src)
                else:
                    nc.vector.tensor_copy(out=vb[mt][:, :, 0:HD], in_=src)

        # ---------------- attention block (hp, ncb)
        class Block:
            def __init__(self, hp, ncb):
                self.hp, self.ncb = hp, ncb
                self.nsl = slice(ncb * NCK, (ncb + 1) * NCK)
                self.pso = None
                self.pts = {}     # mt -> masked-P tile awaiting PV
                self.next_pv = 0
                self.next_step = 0

            def step1(self):
                if self.next_step < MT:
                    self.steps([self.next_step])

            def steps(self, mts):
                mq, mkt = self.hp, 2 + self.hp
                for mt in mts:
                    self.next_step = mt + 1
                    pss = pool2.tile([128, 2, NCK], F32, tag="p2")
                    for par in range(2):
                        po = par * 64
                        nc.tensor.matmul(
                            pss[:, par, :],
                            lhsT=qkT[mkt][mt // 4][po:po + 64,
                                                   (mt % 4) * 128:
                                                   (mt % 4 + 1) * 128],
                            rhs=qkT[mq][self.ncb][po:po + 64, :],
                            start=True, stop=True)
                    pt = ptp.tile([128, 2, NCK], BF16)
                    nc.scalar.activation(
                        out=pt, in_=pss,
                        func=mybir.ActivationFunctionType.Exp, scale=SCALE)
                    pt2 = pt2p.tile([128, 2, NCK], BF16)
                    for par in range(2):
                        nc.vector.tensor_mul(out=pt2[:, par, :],
                                             in0=pt[:, par, :],
                                             in1=mask_sb[mt][:, self.nsl])
                    self.pts[mt] = pt2

            def pvs(self, n):
                if self.pso is None:
                    self.pso = [pool1.tile([128, NCK], F32, tag="p1",
                                           name=f"pso{self.hp}_{self.ncb}_{p}")
                                for p in range(2)]
                for _ in range(n):
                    mt = self.next_pv
                    pt2 = self.pts.pop(mt)
                    for par in range(2):
                        nc.tensor.matmul(
                            self.pso[par][0:HD + 1, :],
                            lhsT=vb[mt][:, 2 * self.hp + par, :],
                            rhs=pt2[:, par, :],
                            start=(mt == 0), stop=(mt == MT - 1))
                    self.next_pv += 1

            def finalize(self):
                for par in range(2):
                    po = par * 64
                    den = dnp.tile([1, NCK], F32, tag="den")
                    nc.vector.tensor_copy(out=den,
                                          in_=self.pso[par][HD:HD + 1, :])
                    rec = dnp.tile([1, NCK], F32, tag="rec")
                    nc.vector.reciprocal_approx_fast(out=rec, in_=den)
                    rb = rbp.tile([64, NCK], F32)
                    nc.gpsimd.partition_broadcast(rb, rec)
                    nc.vector.tensor_mul(
                        out=ot_sb[self.ncb][po:po + 64, self.hp, :],
                        in0=self.pso[par][0:HD, :], in1=rb)

        # ---------------- output projection piece: 2 et rows per call
        def emit_proj_piece(ncb, pair):
            nsl = slice(ncb * NCK, (ncb + 1) * NCK)
            psy = pool2.tile([128, 2, NCK], F32, tag="p2")
            for ee in range(2):
                et = 2 * pair + ee
                for dk in range(2):
                    nc.tensor.matmul(
                        psy[:, ee, :],
                        lhsT=wp_sb[:, dk, et * 128:(et + 1) * 128],
                        rhs=ot_sb[ncb][:, dk, :],
                        start=(dk == 0), stop=(dk == 1))
            yt = ysb.tile([128, 2, NCK], F32)
            nc.vector.tensor_copy(out=yt[:, 0, :], in_=psy[:, 0, :])
            nc.vector.tensor_copy(out=yt[:, 1, :], in_=psy[:, 1, :])
            for ee in range(2):
                et = 2 * pair + ee
                nc.sync.dma_start(out=yT[et * 128:(et + 1) * 128, nsl],
                                  in_=yt[:, ee, :])

        # ---------------- schedule: one block step per phase-A segment
        # (keeps each score's exp dependency a full segment away -> the PE
        # never waits mid-phase-A and holds its high p-state)
        b0 = Block(0, 0)
        b1 = Block(1, 0)
        b01 = Block(0, 1)
        xts = emit_x_dma(0)
        emit_mask([0, 1])
        nc.sync.dma_start(out=wv_sb,
                          in_=wv[:].rearrange("(co ci) d -> ci co d", ci=128))
        emit_warmup(24)
        emit_mask([2, 3])
        emit_A_qk(0, xts, 0)
        b0.step1()
        emit_A_qk(0, xts, 1)
        b0.step1()
        emit_A_v(0, xts, 0)
        b0.step1()
        emit_A_v(0, xts, 1)
        nxts = emit_x_dma(1)
        emit_mask([4, 5, 6, 7])
        xts = nxts
        emit_A_qk(1, xts, 0)
        b0.step1()
        emit_A_qk(1, xts, 1)
        b0.step1()
        emit_A_v(1, xts, 0)
        b0.pvs(2)
        b0.step1()
        emit_A_v(1, xts, 1)
        b1.step1()
        nxts = emit_x_dma(2)
        emit_mask([8, 9, 10, 11])
        xts = nxts
        emit_A_qk(2, xts, 0)
        b0.step1()
        emit_A_qk(2, xts, 1)
        b0.step1()
        b0.pvs(2)
        emit_A_v(2, xts, 0)
        b0.step1()
        emit_A_v(2, xts, 1)
        b1.step1()
        b1.step1()
        b01.step1()
        nxts = emit_x_dma(3)
        emit_mask([12, 13, 14, 15])
        xts = nxts
        emit_A_qk(3, xts, 0)
        b0.step1()
        emit_A_qk(3, xts, 1)
        b0.step1()
        b0.pvs(2)
        emit_A_v(3, xts, 0)
        b0.step1()
        emit_A_v(3, xts, 1)
        b1.step1()
        b0.pvs(2)
        nc.sync.dma_start(out=wp_sb,
                          in_=wp[:].rearrange("(dk ci) e -> ci dk e", ci=128))

        # post-A steady pipeline: step-interleaved boundaries + proj fills
        order = [(1, 0), (0, 1), (1, 1), (0, 2), (1, 2), (0, 3), (1, 3)]
        blocks = {(1, 0): b1, (0, 1): b01}
        prev = b0
        pending_proj = []
        for hp, ncb in order:
            blk = blocks.get((hp, ncb)) or Block(hp, ncb)
            # boundary: prev's last steps alternate with blk's next steps;
            # prev's remaining PVs drain 2 per slot
            for i in range(4):
                prev.step1()
                blk.step1()
                prev.pvs(2)
            # finalize is delayed 3 steps: its DVE ops (normalize TTs
            # waiting on the gpsimd broadcasts) would otherwise head-block
            # the new block's mask stream at the boundary
            for i in range(8):
                blk.step1()
                if i == 2:
                    prev.finalize()
                    if prev.hp == 1:
                        pending_proj += [(prev.ncb, pair)
                                         for pair in range(4)]
                if pending_proj and i in (3, 5, 7):
                    emit_proj_piece(*pending_proj.pop(0))
                if i >= 3:
                    blk.pvs(2 if i <= 5 else 1)
            prev = blk
        # tail: last block steps 12..15, drain, finalize, proj(3)
        for i in range(4):
            prev.step1()
            prev.pvs(2)
        twarm = pool2.tile([128, 2, NCK], F32, tag="p2")
        for i in range(8):
            nc.tensor.matmul(twarm[:, 0, :], lhsT=warm[:, 0:128], rhs=warm,
                             start=True, stop=True)
        prev.finalize()
        for pair in range(4):
            emit_proj_piece(prev.ncb, pair)

    nc.compile()
    return nc


_NC = None


def _get_nc():
    global _NC
    if _NC is None:
        _NC = build()
    return _NC


def make_in_maps(x, mask, W_qkv, W_proj):
    x = np.asarray(x, dtype=np.float32)
    mask = np.asarray(mask)
    W_qkv = np.asarray(W_qkv, dtype=np.float32)
    W_proj = np.asarray(W_proj, dtype=np.float32)
    bf = ml_dtypes.bfloat16
    in_maps = []
    for cid in range(NCORES):
        b, hg = divmod(cid, 4)
        rs = slice(hg * 256, (hg + 1) * 256)
        wq = W_qkv[0 * C:1 * C][rs]          # [256, 1024]
        wk = W_qkv[1 * C:2 * C][rs]
        wvs = W_qkv[2 * C:3 * C][rs]
        in_maps.append({
            "xT": np.ascontiguousarray(x[b].T).astype(bf),
            "wqkT": np.ascontiguousarray(
                np.concatenate([wq, wk], axis=0).T).astype(bf),
            "wvT": np.ascontiguousarray(wvs.T).astype(bf),
            "wpT": np.ascontiguousarray(W_proj[:, rs].T).astype(bf),
            "maskT": np.ascontiguousarray(mask[b, 0].T).astype(bf),
        })
    return in_maps


LAST_EXEC_NS = None
LAST_MEAN_EXEC_NS = None


def kernel(x, mask, W_qkv, W_proj, b_proj):
    global LAST_EXEC_NS, LAST_MEAN_EXEC_NS
    trace = bool(int(os.environ.get("TRNK_TRACE", "0")))
    if trace:
        _ensure_ntff_hook()
    nc = _get_nc()
    in_maps = make_in_maps(x, mask, W_qkv, W_proj)
    res = run_bass_kernel_spmd(nc, in_maps, list(range(NCORES)), trace=trace)
    LAST_EXEC_NS = res.exec_time_ns
    LAST_MEAN_EXEC_NS = res.mean_exec_time_ns
    y = np.zeros((2, N, C), dtype=np.float32)
    for cid in range(NCORES):
        b = cid // 4
        y[b] += np.asarray(res.results[cid]["yT"], dtype=np.float32).T
    y += np.asarray(b_proj, dtype=np.float32)[None, None, :]
    return y


# revision 33
# speedup vs baseline: 1.0249x; 1.0015x over previous
"""Multi-head attention (B=2, N=2048, C=1024, H=16) on 8 Trainium2 cores.

Sharding: core cid = (b, hg) with b = cid//4, hg = cid%4.  Data-parallel on
batch, 4-way tensor-parallel on heads (4 heads / 256 dims per core).  Each
core computes q/k/v projections for its head slice, full (masked-softmax)
attention for its 4 heads, and a partial output projection y^T = Wp_slice^T
-contracted over its 256 dims.  Host sums the 4 partials per batch and adds
the proj bias.

v4 pipeline:
  - Everything bf16 on-device (f32 PSUM accumulation); host pre-casts x, W,
    and the mask (no on-device uint8->bf16 casts).
  - Block (hp, ncb) = 16-m-tile sweep: scores S^T = K^T_h x Q^T_h (head
    pair packed at partition offsets 0/64), exp on Scalar (scale folded),
    mask multiply on DVE into a fresh tile, PV lag-4 behind scores.
  - Step-granular boundary interleave: the last 4 steps of block P
    alternate with the first 4 steps of block B, so the Scalar engine's
    exp stream never drains at block handoffs; P's PV flush + normalize
    land behind B's early steps as PE fill.
  - proj-C psy pairs (2-bank PSUM tiles) spread through the following
    block's steps; psy copy-out split Scalar/DVE.
  - PSUM: pool2 3x2-bank (phase-A segments / score ping-pong / psy pairs)
    + pool1 2x1-bank (O^T accumulators) = 8 banks.
"""

import os
import sys
import types
from contextlib import ExitStack

import numpy as np
import ml_dtypes

import concourse.bass as bass
import concourse.mybir as mybir
import concourse.tile as tile
from concourse import bacc
from concourse.bass_utils import run_bass_kernel_spmd

# ---------------------------------------------------------------- constants
N = 2048          # sequence length
C = 1024          # model dim
NH = 4            # heads per core
HD = 64           # head dim
DQK = 2 * NH * HD # 512: q rows then k rows in qk^T
DV = NH * HD      # 256
NCK = 512         # n-chunk size
NCH = N // NCK    # 4 n-chunks
MT = N // 128     # 16 m-tiles
CK = C // 128     # 8 contraction chunks
SCALE = HD ** -0.5
NCORES = 8

F32 = mybir.dt.float32
BF16 = mybir.dt.bfloat16


def _ensure_ntff_hook():
    """bass_utils' trace path imports antenv.axon_hooks, which this image
    lacks; inject it and register the ctypes-based NTFF profile hook."""
    if "antenv.axon_hooks" in sys.modules:
        return
    mod = types.ModuleType("antenv.axon_hooks")
    _hook = [None]
    mod.set_axon_ntff_profile_hook = lambda h: _hook.__setitem__(0, h)
    mod.get_axon_ntff_profile_hook = lambda: _hook[0]
    sys.modules["antenv.axon_hooks"] = mod
    try:
        from trn_agent_boot.trn_boot import _ntff_profile_via_ctypes

        mod.set_axon_ntff_profile_hook(
            _ntff_profile_via_ctypes("/opt/axon/libaxon_pjrt.so")
        )
    except Exception:
        pass


def build():
    nc = bacc.Bacc("TRN2", target_bir_lowering=False, debug=False,
                   num_devices=NCORES)
    xT = nc.dram_tensor("xT", [C, N], BF16, kind="ExternalInput")
    wqk = nc.dram_tensor("wqkT", [C, DQK], BF16, kind="ExternalInput")
    wv = nc.dram_tensor("wvT", [C, DV], BF16, kind="ExternalInput")
    wp = nc.dram_tensor("wpT", [DV, C], BF16, kind="ExternalInput")
    mk = nc.dram_tensor("maskT", [N, N], BF16, kind="ExternalInput")
    yT = nc.dram_tensor("yT", [C, N], F32, kind="ExternalOutput")

    with tile.TileContext(nc) as tc, ExitStack() as ctx:
        consts = ctx.enter_context(tc.tile_pool(name="consts", bufs=1))
        xin = ctx.enter_context(tc.tile_pool(name="xin", bufs=16))
        ptp = ctx.enter_context(tc.tile_pool(name="ptp", bufs=4))
        pt2p = ctx.enter_context(tc.tile_pool(name="pt2p", bufs=24))
        ysb = ctx.enter_context(tc.tile_pool(name="ysb", bufs=2))
        dnp = ctx.enter_context(tc.tile_pool(name="dnp", bufs=2))
        rbp = ctx.enter_context(tc.tile_pool(name="rbp", bufs=2))
        # PSUM: 3x 2-bank + 2x 1-bank tiles = 8 banks static.
        pool2 = ctx.enter_context(tc.tile_pool(name="pool2", bufs=3, space="PSUM"))
        pool1 = ctx.enter_context(tc.tile_pool(name="pool1", bufs=2, space="PSUM"))

        # ---- resident weights (wp loaded late, after phase-A x traffic)
        wqk_sb = consts.tile([128, CK, DQK], BF16)
        wv_sb = consts.tile([128, CK, DV], BF16)
        wp_sb = consts.tile([128, 2, C], BF16)
        nc.sync.dma_start(out=wqk_sb,
                          in_=wqk[:].rearrange("(co ci) d -> ci co d", ci=128))

        # ---- per-chunk / per-mtile intermediates (fine-grained deps)
        qkT = [[consts.tile([128, NCK], BF16, name=f"qk_m{m}_c{c}")
                for c in range(NCH)] for m in range(4)]
        vb = [consts.tile([128, NH, HD + 1], BF16, name=f"vb_{mt}")
              for mt in range(MT)]
        mask_sb = [consts.tile([128, N], BF16, name=f"mask_m{mt}")
                   for mt in range(MT)]
        ot_sb = [consts.tile([128, 2, NCK], BF16, name=f"ot_n{ncb}")
                 for ncb in range(NCH)]

        # V_aug ones column (during DMA head; engines idle)
        for mt in range(MT):
            nc.gpsimd.memset(vb[mt][:, :, HD:HD + 1], 1.0)

        warm = consts.tile([128, NCK], BF16, name="warm")
        nc.vector.memset(warm[:, 0:NCK], 0.0)

        def emit_warmup(n):
            pwarm = pool1.tile([128, NCK], F32, tag="p1", name="pwarm")
            for i in range(n):
                nc.tensor.matmul(pwarm, lhsT=warm[:, 0:128], rhs=warm,
                                 start=True, stop=True)

        def emit_mask(mts):
            for mt in mts:
                nc.sync.dma_start(out=mask_sb[mt],
                                  in_=mk[mt * 128:(mt + 1) * 128, :])

        def emit_x_dma(c):
            xts = []
            for cc in range(CK):
                xt = xin.tile([128, NCK], BF16)
                nc.sync.dma_start(
                    out=xt[:, 0:NCK // 2],
                    in_=xT[cc * 128:(cc + 1) * 128,
                           c * NCK:c * NCK + NCK // 2])
                nc.sync.dma_start(
                    out=xt[:, NCK // 2:],
                    in_=xT[cc * 128:(cc + 1) * 128,
                           c * NCK + NCK // 2:(c + 1) * NCK])
                xts.append(xt)
            return xts

        # ---------------- phase A: q/k segments (paired so hp=0's Q+K
        # tiles land first), then v segments; PSUM->SBUF copies on
        # Scalar/DVE (early phase, exp stream is light)
        def emit_A_qk(c, xts, pair):
            ms = (0, 2) if pair == 0 else (1, 3)
            pa = pool2.tile([128, 2, NCK], F32, tag="p2")
            for cc in range(CK):
                for mm, m in enumerate(ms):
                    nc.tensor.matmul(
                        pa[:, mm, :],
                        lhsT=wqk_sb[:, cc, m * 128:(m + 1) * 128],
                        rhs=xts[cc], start=(cc == 0), stop=(cc == CK - 1))
            for mm, m in enumerate(ms):
                if mm == 0:
                    nc.scalar.copy(out=qkT[m][c], in_=pa[:, mm, :])
                else:
                    nc.vector.tensor_copy(out=qkT[m][c], in_=pa[:, mm, :])

        def emit_A_v(c, xts, seg):
            pv = pool2.tile([128, 2, NCK], F32, tag="p2")
            for cc in range(CK):
                for jj in range(2):
                    j = 2 * seg + jj
                    nc.tensor.matmul(
                        pv[:, jj, 0:DV],
                        lhsT=xts[cc][:, j * 128:(j + 1) * 128],
                        rhs=wv_sb[:, cc, :],
                        start=(cc == 0), stop=(cc == CK - 1))
            for jj in range(2):
                mt = 4 * c + 2 * seg + jj
                src = pv[:, jj, 0:DV].rearrange("p (h d) -> p h d", h=NH)
                if jj == 0:
                    nc.scalar.copy(out=vb[mt][:, :, 0:HD], in_=src)
                else:
                    nc.vector.tensor_copy(out=vb[mt][:, :, 0:HD], in_=src)

        # ---------------- attention block (hp, ncb)
        class Block:
            def __init__(self, hp, ncb):
                self.hp, self.ncb = hp, ncb
                self.nsl = slice(ncb * NCK, (ncb + 1) * NCK)
                self.pso = None
                self.pts = {}     # mt -> masked-P tile awaiting PV
                self.next_pv = 0
                self.next_step = 0

            def step1(self):
                if self.next_step < MT:
                    self.steps([self.next_step])

            def steps(self, mts):
                mq, mkt = self.hp, 2 + self.hp
                for mt in mts:
                    self.next_step = mt + 1
                    pss = pool2.tile([128, 2, NCK], F32, tag="p2")
                    for par in range(2):
                        po = par * 64
                        nc.tensor.matmul(
                            pss[:, par, :],
                            lhsT=qkT[mkt][mt // 4][po:po + 64,
                                                   (mt % 4) * 128:
                                                   (mt % 4 + 1) * 128],
                            rhs=qkT[mq][self.ncb][po:po + 64, :],
                            start=True, stop=True)
                    pt = ptp.tile([128, 2, NCK], BF16)
                    nc.scalar.activation(
                        out=pt, in_=pss,
                        func=mybir.ActivationFunctionType.Exp, scale=SCALE)
                    pt2 = pt2p.tile([128, 2, NCK], BF16)
                    for par in range(2):
                        nc.vector.tensor_mul(out=pt2[:, par, :],
                                             in0=pt[:, par, :],
                                             in1=mask_sb[mt][:, self.nsl])
                    self.pts[mt] = pt2

            def pvs(self, n):
                if self.pso is None:
                    self.pso = [pool1.tile([128, NCK], F32, tag="p1",
                                           name=f"pso{self.hp}_{self.ncb}_{p}")
                                for p in range(2)]
                for _ in range(n):
                    mt = self.next_pv
                    pt2 = self.pts.pop(mt)
                    for par in range(2):
                        nc.tensor.matmul(
                            self.pso[par][0:HD + 1, :],
                            lhsT=vb[mt][:, 2 * self.hp + par, :],
                            rhs=pt2[:, par, :],
                            start=(mt == 0), stop=(mt == MT - 1))
                    self.next_pv += 1

            def finalize(self):
                for par in range(2):
                    po = par * 64
                    den = dnp.tile([1, NCK], F32, tag="den")
                    nc.vector.tensor_copy(out=den,
                                          in_=self.pso[par][HD:HD + 1, :])
                    rec = dnp.tile([1, NCK], F32, tag="rec")
                    nc.vector.reciprocal_approx_fast(out=rec, in_=den)
                    rb = rbp.tile([64, NCK], F32)
                    nc.gpsimd.partition_broadcast(rb, rec)
                    nc.vector.tensor_mul(
                        out=ot_sb[self.ncb][po:po + 64, self.hp, :],
                        in0=self.pso[par][0:HD, :], in1=rb)

        # ---------------- output projection piece: 2 et rows per call
        def emit_proj_piece(ncb, pair):
            nsl = slice(ncb * NCK, (ncb + 1) * NCK)
            psy = pool2.tile([128, 2, NCK], F32, tag="p2")
            for ee in range(2):
                et = 2 * pair + ee
                for dk in range(2):
                    nc.tensor.matmul(
                        psy[:, ee, :],
                        lhsT=wp_sb[:, dk, et * 128:(et + 1) * 128],
                        rhs=ot_sb[ncb][:, dk, :],
                        start=(dk == 0), stop=(dk == 1))
            yt = ysb.tile([128, 2, NCK], F32)
            nc.vector.tensor_copy(out=yt[:, 0, :], in_=psy[:, 0, :])
            nc.vector.tensor_copy(out=yt[:, 1, :], in_=psy[:, 1, :])
            for ee in range(2):
                et = 2 * pair + ee
                nc.sync.dma_start(out=yT[et * 128:(et + 1) * 128, nsl],
                                  in_=yt[:, ee, :])

        # ---------------- schedule: one block step per phase-A segment
        # (keeps each score's exp dependency a full segment away -> the PE
        # never waits mid-phase-A and holds its high p-state)
        b0 = Block(0, 0)
        b1 = Block(1, 0)
        b01 = Block(0, 1)
        xts = emit_x_dma(0)
        emit_mask([0, 1])
        nc.sync.dma_start(out=wv_sb,
                          in_=wv[:].rearrange("(co ci) d -> ci co d", ci=128))
        emit_warmup(24)
        emit_mask([2, 3])
        emit_A_qk(0, xts, 0)
        b0.step1()
        emit_A_qk(0, xts, 1)
        b0.step1()
        emit_A_v(0, xts, 0)
        b0.step1()
        emit_A_v(0, xts, 1)
        nxts = emit_x_dma(1)
        emit_mask([4, 5, 6, 7])
        xts = nxts
        emit_A_qk(1, xts, 0)
        b0.step1()
        emit_A_qk(1, xts, 1)
        b0.step1()
        emit_A_v(1, xts, 0)
        b0.pvs(2)
        b0.step1()
        emit_A_v(1, xts, 1)
        b1.step1()
        nxts = emit_x_dma(2)
        emit_mask([8, 9, 10, 11])
        xts = nxts
        emit_A_qk(2, xts, 0)
        b0.step1()
        emit_A_qk(2, xts, 1)
        b0.step1()
        b0.pvs(2)
        emit_A_v(2, xts, 0)
        b0.step1()
        emit_A_v(2, xts, 1)
        b1.step1()
        b1.step1()
        b01.step1()
        nxts = emit_x_dma(3)
        emit_mask([12, 13, 14, 15])
        xts = nxts
        emit_A_qk(3, xts, 0)
        b0.step1()
        emit_A_qk(3, xts, 1)
        b0.step1()
        b0.pvs(2)
        emit_A_v(3, xts, 0)
        b0.step1()
        emit_A_v(3, xts, 1)
        b1.step1()
        b0.pvs(2)
        nc.sync.dma_start(out=wp_sb,
                          in_=wp[:].rearrange("(dk ci) e -> ci dk e", ci=128))

        # post-A steady pipeline: step-interleaved boundaries + proj fills
        order = [(1, 0), (0, 1), (1, 1), (0, 2), (1, 2), (0, 3), (1, 3)]
        blocks = {(1, 0): b1, (0, 1): b01}
        prev = b0
        pending_proj = []
        for hp, ncb in order:
            blk = blocks.get((hp, ncb)) or Block(hp, ncb)
            # boundary: prev's last steps alternate with blk's next steps;
            # prev's remaining PVs drain 2 per slot
            for i in range(4):
                prev.step1()
                blk.step1()
                prev.pvs(2)
            # finalize is delayed 3 steps: its DVE ops (normalize TTs
            # waiting on the gpsimd broadcasts) would otherwise head-block
            # the new block's mask stream at the boundary
            for i in range(8):
                blk.step1()
                if i == 2:
                    prev.finalize()
                    if prev.hp == 1:
                        pending_proj += [(prev.ncb, pair)
                                         for pair in range(4)]
                if pending_proj and i in (3, 5, 7):
                    emit_proj_piece(*pending_proj.pop(0))
                if i >= 3:
                    blk.pvs(2 if i <= 5 else 1)
            prev = blk
        # tail: last block steps 12..15, drain, finalize, proj(3)
        for i in range(4):
            prev.step1()
            prev.pvs(2)
        twarm = pool2.tile([128, 2, NCK], F32, tag="p2")
        for i in range(8):
            nc.tensor.matmul(twarm[:, 0, :], lhsT=warm[:, 0:128], rhs=warm,
                             start=True, stop=True)
        prev.finalize()
        for pair in range(4):
            emit_proj_piece(prev.ncb, pair)

    nc.compile()
    return nc


_NC = None


def _get_nc():
    global _NC
    if _NC is None:
        _NC = build()
    return _NC


def make_in_maps(x, mask, W_qkv, W_proj):
    x = np.asarray(x, dtype=np.float32)
    mask = np.asarray(mask)
    W_qkv = np.asarray(W_qkv, dtype=np.float32)
    W_proj = np.asarray(W_proj, dtype=np.float32)
    bf = ml_dtypes.bfloat16
    in_maps = []
    for cid in range(NCORES):
        b, hg = divmod(cid, 4)
        rs = slice(hg * 256, (hg + 1) * 256)
        wq = W_qkv[0 * C:1 * C][rs]          # [256, 1024]
        wk = W_qkv[1 * C:2 * C][rs]
        wvs = W_qkv[2 * C:3 * C][rs]
        in_maps.append({
            "xT": np.ascontiguousarray(x[b].T).astype(bf),
            "wqkT": np.ascontiguousarray(
                np.concatenate([wq, wk], axis=0).T).astype(bf),
            "wvT": np.ascontiguousarray(wvs.T).astype(bf),
            "wpT": np.ascontiguousarray(W_proj[:, rs].T).astype(bf),
            "maskT": np.ascontiguousarray(mask[b, 0].T).astype(bf),
        })
    return in_maps


LAST_EXEC_NS = None
LAST_MEAN_EXEC_NS = None


def kernel(x, mask, W_qkv, W_proj, b_proj):
    global LAST_EXEC_NS, LAST_MEAN_EXEC_NS
    trace = bool(int(os.environ.get("TRNK_TRACE", "0")))
    if trace:
        _ensure_ntff_hook()
    nc = _get_nc()
    in_maps = make_in_maps(x, mask, W_qkv, W_proj)
    res = run_bass_kernel_spmd(nc, in_maps, list(range(NCORES)), trace=trace)
    LAST_EXEC_NS = res.exec_time_ns
    LAST_MEAN_EXEC_NS = res.mean_exec_time_ns
    y = np.zeros((2, N, C), dtype=np.float32)
    for cid in range(NCORES):
        b = cid // 4
        y[b] += np.asarray(res.results[cid]["yT"], dtype=np.float32).T
    y += np.asarray(b_proj, dtype=np.float32)[None, None, :]
    return y
